# revision 12
# baseline (speedup 1.0000x reference)
"""CFG sub-AST expression combiner (segment-softmax scatter attention) on 8 trn2 cores.

Strategy: sort edges by segment (PDG node); assign 16-segment windows to cores
round-robin by descending edge count (load balance, softmax stays core-local).
Host folds Wq/Wk into a per-segment vector table C = A @ (Wq Wk^T)/sqrt(d) and
pre-gathers edge value rows into two DRAM layouts (slot-major V and transposed
V^T), so the device streams everything with large contiguous DMAs. The segment
mask is additive pre-exp: a K=16 one-hot matmul adds +M to each edge's own
segment column, so after exp the wrong columns are suppressed by e^-M and no
per-element mask multiply is needed. Pipeline per window:
scores = V @ C_window^T + M*onehot (PE) -> exp (ACT) -> PV + denominator
matmuls (PE, PSUM-accumulated, two windows per PSUM bank) -> divide (DVE) ->
project with Wo (PE). Empty segments are patched to bias-only on host.
"""

import math

import numpy as np
import ml_dtypes

import concourse.bass as bass
from concourse import bacc
import concourse.mybir as mybir
from concourse.bass_types import AP
from concourse.tile import TileContext, add_dep_helper
from concourse import bass_utils

BF16 = ml_dtypes.bfloat16
N_CORES = 8
D = 128          # feature dim
H = 8            # heads
W = 16           # segment window width (output columns per score matmul)
HW = H * W       # score columns per tile (128)
P = 128          # edge slots per tile (partition dim)
PROJ_B = 8       # windows per output-projection batch (PROJ_B*W = 128 cols)
TC = 64          # max tiles per chunk
MBOOST = 45.0    # additive score boost for an edge's own segment column
F32 = mybir.dt.float32
BF = mybir.dt.bfloat16


def _build_nc(NW, T_ws, chunks, G_max, T_max, comb):
    """One SPMD program for all cores. NW windows of W segments; window j owns
    T_ws[j] tiles of P edge slots (uniform across cores). chunks: list of
    (j0, j1, o0, o1) slot/tile ranges streamed together."""
    S_pad = NW * W
    S_t = sum(T_ws)
    assert NW % PROJ_B == 0 and NW % 2 == 0
    nc = bacc.Bacc("TRN2", target_bir_lowering=False)

    ev = nc.dram_tensor("ev", [P, S_t * D], BF, kind="ExternalInput")
    evt = nc.dram_tensor("evt", [D, S_t * P], BF, kind="ExternalInput")
    cc = nc.dram_tensor("cc", [D, NW * HW], BF, kind="ExternalInput")
    oh = nc.dram_tensor("oh", [W, S_t * P], BF, kind="ExternalInput")
    cb = nc.dram_tensor("cb", [W, HW], BF, kind="ExternalInput")
    wo = nc.dram_tensor("wo", [D, H * comb], BF, kind="ExternalInput")
    bo = nc.dram_tensor("bo", [comb, 1], F32, kind="ExternalInput")
    out = nc.dram_tensor("out", [comb, S_pad], F32, kind="ExternalOutput")

    EXP = mybir.ActivationFunctionType.Exp

    with TileContext(nc) as tc:
        with (
            tc.tile_pool(name="const", bufs=1) as constp,
            tc.tile_pool(name="ccp", bufs=2) as ccp,
            tc.tile_pool(name="ohp", bufs=2) as ohp,
            tc.tile_pool(name="vg", bufs=2) as vgp,
            tc.tile_pool(name="vt", bufs=2) as vtp,
            tc.tile_pool(name="sx", bufs=2) as sxp,
            tc.tile_pool(name="den", bufs=2) as denp,
            tc.tile_pool(name="rec", bufs=2) as recp,
            tc.tile_pool(name="hot", bufs=2) as hotp,
            tc.tile_pool(name="ps_s", bufs=2, space="PSUM") as ps_s,
            tc.tile_pool(name="ps_acc", bufs=2, space="PSUM") as ps_acc,
            tc.tile_pool(name="ps_bc", bufs=2, space="PSUM") as ps_bc,
            tc.tile_pool(name="ps_ops", bufs=2, space="PSUM") as ps_ops,
        ):
            # ---- preload constants (HWDGE on SP) ----
            wo_sb = constp.tile([D, H, comb], BF, tag="wo")
            nc.sync.dma_start(wo_sb[:], wo[:].rearrange("d (h c) -> d h c", h=H))
            bo_sb = constp.tile([comb, 1], F32, tag="bo")
            nc.sync.dma_start(bo_sb[:], bo[:])
            cb_sb = constp.tile([W, HW], BF, tag="cb")
            nc.sync.dma_start(cb_sb[:], cb[:])
            ones_col = constp.tile([P, 1], BF, tag="ones_col")
            nc.vector.memset(ones_col[:], 1.0)
            ones_row = constp.tile([1, P], F32, tag="ones_row")
            nc.vector.memset(ones_row[:], 1.0)
            outb = constp.tile([comb, S_pad], F32, tag="outb")

            n_ch = len(chunks)
            cc_t = [None] * n_ch
            oh_t = [None] * n_ch
            vg_t = [None] * n_ch
            vt_t = [None] * n_ch

            def issue_chunk(k):
                j0, j1, o0, o1 = chunks[k]
                Tc, G = o1 - o0, j1 - j0
                vg_t[k] = vgp.tile([P, TC * D], BF, tag="vg", name="vgt")
                nc.sync.dma_start(vg_t[k][:, 0:Tc * D], ev[:, o0 * D:o1 * D])
                vt_t[k] = vtp.tile([D, TC * P], BF, tag="vt", name="vtt")
                nc.sync.dma_start(vt_t[k][:, 0:Tc * P], evt[:, o0 * P:o1 * P])
                cc_t[k] = ccp.tile([D, G_max * HW], BF, tag="cc", name="cct")
                nc.sync.dma_start(cc_t[k][:, 0:G * HW], cc[:, j0 * HW:j1 * HW])
                oh_t[k] = ohp.tile([W, TC * P], BF, tag="oh", name="oht")
                nc.sync.dma_start(oh_t[k][:, 0:Tc * P], oh[:, o0 * P:o1 * P])

            issue_chunk(0)

            # per-window geometry: (chunk k, local tile offset, local window idx)
            geom = []
            for k, (j0, j1, o0, o1) in enumerate(chunks):
                ol = 0
                for j in range(j0, j1):
                    geom.append((k, ol, j - j0))
                    ol += T_ws[j]

            s_tiles = [None] * NW

            def emit_scores(j):
                k, ol, jc = geom[j]
                T_w = T_ws[j]
                s_ps = ps_s.tile([P, T_max, HW], F32, tag="s", name="s_ps")
                s_tiles[j] = s_ps
                for t in range(T_w):
                    nc.tensor.matmul(
                        s_ps[:, t, :],
                        lhsT=vt_t[k][:, (ol + t) * P:(ol + t + 1) * P],
                        rhs=cc_t[k][:, jc * HW:(jc + 1) * HW],
                        start=True,
                        stop=False,
                    )
                    nc.tensor.matmul(
                        s_ps[:, t, :],
                        lhsT=oh_t[k][:, (ol + t) * P:(ol + t + 1) * P],
                        rhs=cb_sb[:],
                        start=False,
                        stop=True,
                    )

            acc = None
            hot = None
            pv_last = None
            dn_last = None

            def emit_rest(j):
                nonlocal acc, hot, pv_last, dn_last
                k, ol, jc = geom[j]
                T_w = T_ws[j]
                wp = j % 2  # position within psum pair
                s_ps = s_tiles[j]
                # exp (mask already folded in additively)
                sx = sxp.tile([P, T_max, H, W], BF, tag="sx")
                nc.scalar.activation(
                    sx[:, 0:T_w].rearrange("p t h w -> p t (h w)"),
                    s_ps[:, 0:T_w, :],
                    EXP,
                )
                # two windows share one PSUM bank:
                # cols [wp*HW, (wp+1)*HW) = pv, [2*HW + wp*HW, ...) = dn
                if wp == 0:
                    acc = ps_acc.tile([P, 4 * HW], F32, tag="acc", name="acc")
                    pv_last = dn_last = None
                for t in range(T_w):
                    mm = nc.tensor.matmul(
                        acc[:, wp * HW:(wp + 1) * HW],
                        lhsT=vg_t[k][:, (ol + t) * D:(ol + t + 1) * D],
                        rhs=sx[:, t, :, :].rearrange("p h w -> p (h w)"),
                        start=(t == 0),
                        stop=(t == T_w - 1),
                    )
                    if t == 0 and pv_last is not None:
                        add_dep_helper(mm.ins, pv_last.ins,
                                       reason="pv group order in shared bank")
                    pv_last = mm
                for t in range(T_w):
                    mm = nc.tensor.matmul(
                        acc[0:1, (2 + wp) * HW:(3 + wp) * HW],
                        lhsT=ones_col[:],
                        rhs=sx[:, t, :, :].rearrange("p h w -> p (h w)"),
                        start=(t == 0),
                        stop=(t == T_w - 1),
                    )
                    if t == 0:
                        add_dep_helper(mm.ins, pv_last.ins,
                                       reason="dn group after pv group")
                        if dn_last is not None:
                            add_dep_helper(mm.ins, dn_last.ins,
                                           reason="dn group order in shared bank")
                    dn_last = mm
                if wp == 1:
                    # normalize both windows of the pair at once
                    den = denp.tile([1, 2 * HW], F32, tag="den")
                    nc.vector.tensor_scalar_add(
                        den[:], acc[0:1, 2 * HW:4 * HW], 1e-30)
                    bc_ps = ps_bc.tile([P, 2 * HW], F32, tag="bc", name="bc_ps")
                    bc_mm = nc.tensor.matmul(
                        bc_ps[:], lhsT=ones_row[:], rhs=den[:],
                        start=True, stop=True)
                    add_dep_helper(bc_mm.ins, dn_last.ins,
                                   reason="bc after dn groups")
                    rec = recp.tile([P, 2 * HW], F32, tag="rec")
                    nc.vector.reciprocal(rec[:], bc_ps[:])
                    jj = j % PROJ_B  # 1, 3, 5, 7
                    if jj == 1:
                        hot = hotp.tile([P, PROJ_B, H, W], BF, tag="hot",
                                        name="hot")
                    nc.vector.tensor_mul(
                        hot[:, jj - 1:jj + 1].rearrange("p b h w -> p (b h w)"),
                        acc[:, 0:2 * HW], rec[:],
                    )
                # output projection every PROJ_B windows
                if j % PROJ_B == PROJ_B - 1:
                    jbase = j - (PROJ_B - 1)
                    ops = ps_ops.tile([comb, PROJ_B * W], F32, tag="ops",
                                      name="ops")
                    for h in range(H):
                        nc.tensor.matmul(
                            ops[:].rearrange("c (b w) -> c b w", b=PROJ_B),
                            lhsT=wo_sb[:, h, :],
                            rhs=hot[:, :, h, :],
                            start=(h == 0),
                            stop=(h == H - 1),
                        )
                    nc.vector.tensor_scalar_add(
                        outb[:, jbase * W:(j + 1) * W], ops[:], bo_sb[:]
                    )

            # software pipeline: scores one window ahead of the rest
            emit_scores(0)
            for k in range(n_ch):
                if k + 1 < n_ch:
                    issue_chunk(k + 1)
                j0, j1 = chunks[k][0], chunks[k][1]
                for j in range(j0, j1):
                    if j + 1 < NW:
                        emit_scores(j + 1)
                    emit_rest(j)

            nc.sync.dma_start(out[:], outb[:])
    nc.compile()
    return nc


def _plan(ast_value, N):
    """Window/tile structure + per-edge slot assignment (core, partition, tile)."""
    E = ast_value.shape[0]
    NWg = -(-N // W)               # global window count
    order = np.argsort(ast_value, kind="stable")
    seg_s = ast_value[order].astype(np.int64)
    win_s = seg_s // W

    n_w = np.bincount(win_s, minlength=NWg)
    t_w = np.maximum(1, -(-n_w // P))
    # round-robin by descending edge count -> near-equal per-core tile budgets
    wrank = np.argsort(-n_w, kind="stable")
    core_of_w = np.empty(NWg, np.int64)
    slot_of_w = np.empty(NWg, np.int64)
    core_of_w[wrank] = np.arange(NWg) % N_CORES
    slot_of_w[wrank] = np.arange(NWg) // N_CORES
    NW = -(-NWg // N_CORES)
    NW = -(-NW // PROJ_B) * PROJ_B
    # shared (max-over-octet) tile counts per slot; wrank sorted desc => rank 8j
    T_ws = np.ones(NW, np.int64)
    T_ws[: (NWg + N_CORES - 1) // N_CORES] = t_w[wrank[0::N_CORES]]
    tile_off = np.zeros(NW + 1, np.int64)
    np.cumsum(T_ws, out=tile_off[1:])
    S_t = int(tile_off[-1])

    starts = np.zeros(NWg, np.int64)
    np.cumsum(n_w[:-1], out=starts[1:])
    rank_e = np.arange(E, dtype=np.int64) - starts[win_s]
    t_e = rank_e // P
    p_e = rank_e % P
    core_e = core_of_w[win_s]
    g_e = tile_off[slot_of_w[win_s]] + t_e
    return (order, seg_s, win_s, n_w, core_of_w, slot_of_w, NW, T_ws, tile_off,
            S_t, core_e, p_e, g_e)


def _run(ast, Wq, bq, Wk, bk, Wo, bo, ast_key, ast_value, pdg_key, pdg_value, N,
         trace=False):
    """Host orchestration: build plan from data, compile, run on 8 cores."""
    n_tbl, d = ast.shape
    assert d == D
    comb = Wo.shape[1]
    sc = 1.0 / math.sqrt(D)

    (order, seg_s, win_s, n_w, core_of_w, slot_of_w, NW, T_ws, tile_off, S_t,
     core_e, p_e, g_e) = _plan(ast_value, N)
    key_s = ast_key[order].astype(np.int64)
    NWg = -(-N // W)

    # host pre-gather: slot (core, p, tile) -> value row, in both layouts.
    # unused slots point at a zero row -> they contribute exp(0)*0 to PV and
    # a negligible exp(0)=1 to the denominator.
    tblz = np.vstack([ast.astype(BF16), np.zeros((1, D), BF16)])
    gidx_all = np.full((N_CORES, P, S_t), n_tbl, np.int64)
    gidx_all[core_e, p_e, g_e] = key_s
    ev_all = tblz[gidx_all.reshape(N_CORES, -1)]          # [8, P*S_t, D]
    ev_all = ev_all.reshape(N_CORES, P, S_t * D)
    evt_all = np.ascontiguousarray(
        ev_all.reshape(N_CORES, P, S_t, D).transpose(0, 3, 2, 1)
    ).reshape(N_CORES, D, S_t * P)

    # additive one-hot mask operand [W, S_t*P] (lhsT of the K=W boost matmul)
    oh_f = np.zeros((N_CORES, W, S_t * P), np.float32)
    oh_f[core_e, seg_s % W, g_e * P + p_e] = 1.0
    oh_all = oh_f.astype(BF16)
    cb_arr = (np.eye(W, dtype=np.float32)[:, None, :]
              * np.float32(MBOOST)).repeat(H, axis=1).reshape(W, HW)
    cb_arr = np.ascontiguousarray(cb_arr).astype(BF16)

    # ---- query-side fold: C = A @ (Wq' Wk^T) + bq' @ Wk^T ----
    qsrc = np.zeros(N, np.int64)
    qsrc[pdg_key.astype(np.int64)] = pdg_value.astype(np.int64)
    A = ast[qsrc]                                        # [N, D] f32
    M = np.einsum("hij,hkj->hik", Wq * sc, Wk)           # [H, D, D]
    kap = np.einsum("hj,hkj->hk", bq * sc, Wk)           # [H, D]
    C8 = np.einsum("nd,hdk->hnk", A, M) + kap[:, None, :]  # [H, N, D]

    # per-core window lists -> cc layout [D, NW*H*W]
    wl = np.full((N_CORES, NW), -1, np.int64)
    wl[core_of_w, slot_of_w] = np.arange(NWg)
    seg_raw = wl[:, :, None] * W + np.arange(W)[None, None, :]  # [8, NW, W]
    valid = (wl[:, :, None] >= 0) & (seg_raw < N)
    seg_ids = np.clip(seg_raw, 0, N - 1)
    ccv = C8[:, seg_ids, :]                              # [H, 8, NW, W, D]
    cc_all = np.ascontiguousarray(
        ccv.transpose(1, 4, 2, 0, 3)                     # [8, D, NW, H, W]
    ).astype(BF16).reshape(N_CORES, D, NW * HW)

    # chunks of consecutive slots with <= TC tiles
    chunks = []
    j0 = 0
    while j0 < NW:
        j1 = j0
        while j1 < NW and tile_off[j1 + 1] - tile_off[j0] <= TC:
            j1 += 1
        chunks.append((j0, j1, int(tile_off[j0]), int(tile_off[j1])))
        j0 = j1
    G_max = max(j1 - j0 for j0, j1, _, _ in chunks)
    T_max = int(T_ws.max())

    wo_arr = np.ascontiguousarray(
        Wo.reshape(H, D, comb).transpose(1, 0, 2)
    ).astype(BF16).reshape(D, H * comb)
    bo_col = bo.reshape(comb, 1).astype(np.float32)

    nc = _build_nc(NW, [int(x) for x in T_ws], chunks, G_max, T_max, comb)
    in_maps = []
    for c in range(N_CORES):
        in_maps.append({
            "ev": ev_all[c],
            "evt": evt_all[c],
            "cc": cc_all[c],
            "oh": oh_all[c],
            "cb": cb_arr,
            "wo": wo_arr,
            "bo": bo_col,
        })
    res = bass_utils.run_bass_kernel_spmd(
        nc, in_maps, core_ids=list(range(N_CORES)), trace=trace
    )
    full = np.zeros((N, comb), np.float32)
    for c in range(N_CORES):
        outc = np.asarray(res.results[c]["out"], np.float32).T  # [S_pad, comb]
        vm = valid[c].reshape(-1)
        sel = seg_ids[c].reshape(-1)[vm]
        full[sel] = outc[: vm.shape[0]][vm]
    # empty segments: reference = bias only (suppression garbage otherwise)
    seg_cnt = np.bincount(ast_value.astype(np.int64), minlength=N)[:N]
    full[seg_cnt == 0] = bo[None, :]
    return full, res


def kernel(**inputs):
    ast = np.asarray(inputs["ast_nodes_encodings"], np.float32)
    Wq = np.asarray(inputs["Wq"], np.float32)
    bq = np.asarray(inputs["bq"], np.float32)
    Wk = np.asarray(inputs["Wk"], np.float32)
    bk = np.asarray(inputs["bk"], np.float32)  # cancels inside segment softmax
    Wo = np.asarray(inputs["Wo"], np.float32)
    bo = np.asarray(inputs["bo"], np.float32)
    ast_key = np.asarray(inputs["ast_key"]).astype(np.int64)
    ast_value = np.asarray(inputs["ast_value"]).astype(np.int64)
    pdg_key = np.asarray(inputs["pdg_key"]).astype(np.int64)
    pdg_value = np.asarray(inputs["pdg_value"]).astype(np.int64)
    N = int(np.asarray(inputs["nr_cfg_nodes"]))
    out, _ = _run(ast, Wq, bq, Wk, bk, Wo, bo,
                  ast_key, ast_value, pdg_key, pdg_value, N)
    return out


# revision 13
# speedup vs baseline: 3.1149x; 3.1149x over previous
"""CFG sub-AST expression combiner (segment-softmax scatter attention) on 8 trn2 cores.

Strategy: sort edges by segment (PDG node); assign 16-segment windows to cores
round-robin by descending edge count (load balance, softmax stays core-local).
Host folds Wq/Wk into a per-segment vector table C = A @ (Wq Wk^T)/sqrt(d) and
pre-gathers edge value rows into two DRAM layouts (slot-major V and transposed
V^T), so the device streams everything with large contiguous DMAs. Pipeline per
window: scores = V @ C_window^T (PE) -> exp (ACT) -> mask (DVE) -> PV matmuls +
broadcast-denominator matmul (all-ones stationary; PSUM-accumulated, two
windows per PSUM bank) -> fast-reciprocal + normalize (DVE) -> project with Wo
(PE). Empty segments are patched to bias-only on host.
"""

import math

import numpy as np
import ml_dtypes

import concourse.bass as bass
from concourse import bacc
import concourse.mybir as mybir
from concourse.bass_types import AP
from concourse.tile import TileContext, add_dep_helper
from concourse import bass_utils

BF16 = ml_dtypes.bfloat16
N_CORES = 8
D = 128          # feature dim
H = 8            # heads
W = 16           # segment window width (output columns per score matmul)
HW = H * W       # score columns per tile (128)
P = 128          # edge slots per tile (partition dim)
PROJ_B = 8       # windows per output-projection batch (PROJ_B*W = 128 cols)
TC = 64          # max tiles per chunk
F32 = mybir.dt.float32
BF = mybir.dt.bfloat16


def _build_nc(NW, T_ws, chunks, G_max, T_max, comb):
    """One SPMD program for all cores. NW windows of W segments; window j owns
    T_ws[j] tiles of P edge slots (uniform across cores). chunks: list of
    (j0, j1, o0, o1) slot/tile ranges streamed together."""
    S_pad = NW * W
    S_t = sum(T_ws)
    assert NW % PROJ_B == 0 and NW % 2 == 0
    nc = bacc.Bacc("TRN2", target_bir_lowering=False)

    ev = nc.dram_tensor("ev", [P, S_t * D], BF, kind="ExternalInput")
    evt = nc.dram_tensor("evt", [D, S_t * P], BF, kind="ExternalInput")
    cc = nc.dram_tensor("cc", [D, NW * HW], BF, kind="ExternalInput")
    msk = nc.dram_tensor("msk", [P, S_t * W], BF, kind="ExternalInput")
    wo = nc.dram_tensor("wo", [D, H * comb], BF, kind="ExternalInput")
    bo = nc.dram_tensor("bo", [comb, 1], F32, kind="ExternalInput")
    out = nc.dram_tensor("out", [comb, S_pad], F32, kind="ExternalOutput")

    EXP = mybir.ActivationFunctionType.Exp

    with TileContext(nc) as tc:
        with (
            tc.tile_pool(name="const", bufs=1) as constp,
            tc.tile_pool(name="ccp", bufs=2) as ccp,
            tc.tile_pool(name="mkp", bufs=2) as mkp,
            tc.tile_pool(name="vg", bufs=2) as vgp,
            tc.tile_pool(name="vt", bufs=2) as vtp,
            tc.tile_pool(name="sx", bufs=3) as sxp,
            tc.tile_pool(name="pt", bufs=3) as ptp,
            tc.tile_pool(name="rec", bufs=2) as recp,
            tc.tile_pool(name="hot", bufs=2) as hotp,
            tc.tile_pool(name="ps_s", bufs=3, space="PSUM") as ps_s,
            tc.tile_pool(name="ps_acc", bufs=2, space="PSUM") as ps_acc,
            tc.tile_pool(name="ps_ops", bufs=2, space="PSUM") as ps_ops,
        ):
            # ---- preload constants (HWDGE on SP) ----
            wo_sb = constp.tile([D, H, comb], BF, tag="wo")
            nc.sync.dma_start(wo_sb[:], wo[:].rearrange("d (h c) -> d h c", h=H))
            bo_sb = constp.tile([comb, 1], F32, tag="bo")
            nc.sync.dma_start(bo_sb[:], bo[:])
            ones_mat = constp.tile([P, P], BF, tag="ones_mat")
            nc.vector.memset(ones_mat[:], 1.0)
            outb = constp.tile([comb, S_pad], F32, tag="outb")

            n_ch = len(chunks)
            cc_t = [None] * n_ch
            mk_t = [None] * n_ch
            vg_t = [None] * n_ch
            vt_t = [None] * n_ch

            def issue_chunk(k):
                j0, j1, o0, o1 = chunks[k]
                Tc, G = o1 - o0, j1 - j0
                vg_t[k] = vgp.tile([P, TC * D], BF, tag="vg", name="vgt")
                nc.sync.dma_start(vg_t[k][:, 0:Tc * D], ev[:, o0 * D:o1 * D])
                vt_t[k] = vtp.tile([D, TC * P], BF, tag="vt", name="vtt")
                nc.sync.dma_start(vt_t[k][:, 0:Tc * P], evt[:, o0 * P:o1 * P])
                cc_t[k] = ccp.tile([D, G_max * HW], BF, tag="cc", name="cct")
                nc.sync.dma_start(cc_t[k][:, 0:G * HW], cc[:, j0 * HW:j1 * HW])
                mk_t[k] = mkp.tile([P, TC * W], BF, tag="mk", name="mkt")
                nc.sync.dma_start(mk_t[k][:, 0:Tc * W], msk[:, o0 * W:o1 * W])

            issue_chunk(0)

            # per-window geometry: (chunk k, local tile offset, local window idx)
            geom = []
            for k, (j0, j1, o0, o1) in enumerate(chunks):
                ol = 0
                for j in range(j0, j1):
                    geom.append((k, ol, j - j0))
                    ol += T_ws[j]

            s_tiles = [None] * NW

            def emit_scores(j):
                k, ol, jc = geom[j]
                T_w = T_ws[j]
                s_ps = ps_s.tile([P, T_max, HW], F32, tag="s", name="s_ps")
                s_tiles[j] = s_ps
                for t in range(T_w):
                    nc.tensor.matmul(
                        s_ps[:, t, :],
                        lhsT=vt_t[k][:, (ol + t) * P:(ol + t + 1) * P],
                        rhs=cc_t[k][:, jc * HW:(jc + 1) * HW],
                        start=True,
                        stop=True,
                    )

            acc = None
            hot = None
            pv_last = None
            dn_last = None

            def emit_rest(j):
                nonlocal acc, hot, pv_last, dn_last
                k, ol, jc = geom[j]
                T_w = T_ws[j]
                wp = j % 2  # position within psum pair
                s_ps = s_tiles[j]
                # exp then mask (mask broadcast over heads)
                sx = sxp.tile([P, T_max, H, W], BF, tag="sx")
                nc.scalar.activation(
                    sx[:, 0:T_w].rearrange("p t h w -> p t (h w)"),
                    s_ps[:, 0:T_w, :],
                    EXP,
                )
                pt = ptp.tile([P, T_max, H, W], BF, tag="pt")
                mv = mk_t[k][:, ol * W:(ol + T_w) * W].rearrange(
                    "p (t w) -> p t w", w=W)
                mb = AP(mv.tensor, mv.offset,
                        [mv.ap[0], mv.ap[1], [0, H], mv.ap[2]])
                nc.vector.tensor_mul(pt[:, 0:T_w], sx[:, 0:T_w], mb)
                # two windows share one PSUM bank:
                # cols [wp*HW, (wp+1)*HW) = pv, [(2+wp)*HW, (3+wp)*HW) = dn
                # (dn uses an all-ones stationary -> denominator replicated
                # across all 128 partitions, no separate broadcast matmul)
                if wp == 0:
                    acc = ps_acc.tile([P, 4 * HW], F32, tag="acc", name="acc")
                    pv_last = dn_last = None
                for t in range(T_w):
                    mm = nc.tensor.matmul(
                        acc[:, wp * HW:(wp + 1) * HW],
                        lhsT=vg_t[k][:, (ol + t) * D:(ol + t + 1) * D],
                        rhs=pt[:, t, :, :].rearrange("p h w -> p (h w)"),
                        start=(t == 0),
                        stop=(t == T_w - 1),
                    )
                    if t == 0 and pv_last is not None:
                        add_dep_helper(mm.ins, pv_last.ins,
                                       reason="pv group order in shared bank")
                    pv_last = mm
                for t in range(T_w):
                    mm = nc.tensor.matmul(
                        acc[:, (2 + wp) * HW:(3 + wp) * HW],
                        lhsT=ones_mat[:],
                        rhs=pt[:, t, :, :].rearrange("p h w -> p (h w)"),
                        start=(t == 0),
                        stop=(t == T_w - 1),
                    )
                    if t == 0:
                        add_dep_helper(mm.ins, pv_last.ins,
                                       reason="dn group after pv group")
                        if dn_last is not None:
                            add_dep_helper(mm.ins, dn_last.ins,
                                           reason="dn group order in shared bank")
                    dn_last = mm
                if wp == 1:
                    # normalize both windows of the pair at once
                    rec = recp.tile([P, 2 * HW], F32, tag="rec")
                    nc.vector.reciprocal_approx_fast(rec[:], acc[:, 2 * HW:4 * HW])
                    jj = j % PROJ_B  # 1, 3, 5, 7
                    if jj == 1:
                        hot = hotp.tile([P, PROJ_B, H, W], BF, tag="hot",
                                        name="hot")
                    nc.vector.tensor_mul(
                        hot[:, jj - 1:jj + 1].rearrange("p b h w -> p (b h w)"),
                        acc[:, 0:2 * HW], rec[:],
                    )
                # output projection every PROJ_B windows
                if j % PROJ_B == PROJ_B - 1:
                    jbase = j - (PROJ_B - 1)
                    ops = ps_ops.tile([comb, PROJ_B * W], F32, tag="ops",
                                      name="ops")
                    for h in range(H):
                        nc.tensor.matmul(
                            ops[:].rearrange("c (b w) -> c b w", b=PROJ_B),
                            lhsT=wo_sb[:, h, :],
                            rhs=hot[:, :, h, :],
                            start=(h == 0),
                            stop=(h == H - 1),
                        )
                    nc.vector.tensor_scalar_add(
                        outb[:, jbase * W:(j + 1) * W], ops[:], bo_sb[:]
                    )

            # software pipeline: scores two windows ahead of the rest
            LOOKAHEAD = 2
            for j in range(min(LOOKAHEAD, NW)):
                emit_scores(j)
            for k in range(n_ch):
                if k + 1 < n_ch:
                    issue_chunk(k + 1)
                j0, j1 = chunks[k][0], chunks[k][1]
                for j in range(j0, j1):
                    if j + LOOKAHEAD < NW:
                        emit_scores(j + LOOKAHEAD)
                    emit_rest(j)

            nc.sync.dma_start(out[:], outb[:])
    nc.compile()
    return nc


def _plan(ast_value, N):
    """Window/tile structure + per-edge slot assignment (core, partition, tile)."""
    E = ast_value.shape[0]
    NWg = -(-N // W)               # global window count
    order = np.argsort(ast_value, kind="stable")
    seg_s = ast_value[order].astype(np.int64)
    win_s = seg_s // W

    n_w = np.bincount(win_s, minlength=NWg)
    t_w = np.maximum(1, -(-n_w // P))
    # round-robin by descending edge count -> near-equal per-core tile budgets
    wrank = np.argsort(-n_w, kind="stable")
    core_of_w = np.empty(NWg, np.int64)
    slot_of_w = np.empty(NWg, np.int64)
    core_of_w[wrank] = np.arange(NWg) % N_CORES
    slot_of_w[wrank] = np.arange(NWg) // N_CORES
    NW = -(-NWg // N_CORES)
    NW = -(-NW // PROJ_B) * PROJ_B
    # shared (max-over-octet) tile counts per slot; wrank sorted desc => rank 8j
    T_ws = np.ones(NW, np.int64)
    T_ws[: (NWg + N_CORES - 1) // N_CORES] = t_w[wrank[0::N_CORES]]
    tile_off = np.zeros(NW + 1, np.int64)
    np.cumsum(T_ws, out=tile_off[1:])
    S_t = int(tile_off[-1])

    starts = np.zeros(NWg, np.int64)
    np.cumsum(n_w[:-1], out=starts[1:])
    rank_e = np.arange(E, dtype=np.int64) - starts[win_s]
    t_e = rank_e // P
    p_e = rank_e % P
    core_e = core_of_w[win_s]
    g_e = tile_off[slot_of_w[win_s]] + t_e
    return (order, seg_s, win_s, core_of_w, slot_of_w, NW, T_ws, tile_off,
            S_t, core_e, p_e, g_e)


def _run(ast, Wq, bq, Wk, bk, Wo, bo, ast_key, ast_value, pdg_key, pdg_value, N,
         trace=False):
    """Host orchestration: build plan from data, compile, run on 8 cores."""
    n_tbl, d = ast.shape
    assert d == D
    comb = Wo.shape[1]
    sc = 1.0 / math.sqrt(D)

    (order, seg_s, win_s, core_of_w, slot_of_w, NW, T_ws, tile_off, S_t,
     core_e, p_e, g_e) = _plan(ast_value, N)
    key_s = ast_key[order].astype(np.int64)
    NWg = -(-N // W)

    # host pre-gather: slot (core, p, tile) -> value row, in both layouts.
    tblz = np.vstack([ast.astype(BF16), np.zeros((1, D), BF16)])
    gidx_all = np.full((N_CORES, P, S_t), n_tbl, np.int64)
    gidx_all[core_e, p_e, g_e] = key_s
    ev_all = tblz[gidx_all.reshape(N_CORES, -1)]          # [8, P*S_t, D]
    ev_all = ev_all.reshape(N_CORES, P, S_t * D)
    evt_all = np.ascontiguousarray(
        ev_all.reshape(N_CORES, P, S_t, D).transpose(0, 3, 2, 1)
    ).reshape(N_CORES, D, S_t * P)

    msk_f = np.zeros((N_CORES, P, S_t * W), np.float32)
    msk_f[core_e, p_e, g_e * W + seg_s % W] = 1.0
    msk_all = msk_f.astype(BF16)

    # ---- query-side fold: C = A @ (Wq' Wk^T) + bq' @ Wk^T ----
    qsrc = np.zeros(N, np.int64)
    qsrc[pdg_key.astype(np.int64)] = pdg_value.astype(np.int64)
    A = ast[qsrc]                                        # [N, D] f32
    M = np.einsum("hij,hkj->hik", Wq * sc, Wk)           # [H, D, D]
    kap = np.einsum("hj,hkj->hk", bq * sc, Wk)           # [H, D]
    C8 = np.einsum("nd,hdk->hnk", A, M) + kap[:, None, :]  # [H, N, D]

    # per-core window lists -> cc layout [D, NW*H*W]
    wl = np.full((N_CORES, NW), -1, np.int64)
    wl[core_of_w, slot_of_w] = np.arange(NWg)
    seg_raw = wl[:, :, None] * W + np.arange(W)[None, None, :]  # [8, NW, W]
    valid = (wl[:, :, None] >= 0) & (seg_raw < N)
    seg_ids = np.clip(seg_raw, 0, N - 1)
    ccv = C8[:, seg_ids, :]                              # [H, 8, NW, W, D]
    cc_all = np.ascontiguousarray(
        ccv.transpose(1, 4, 2, 0, 3)                     # [8, D, NW, H, W]
    ).astype(BF16).reshape(N_CORES, D, NW * HW)

    # chunks of consecutive slots with <= TC tiles
    chunks = []
    j0 = 0
    while j0 < NW:
        j1 = j0
        while j1 < NW and tile_off[j1 + 1] - tile_off[j0] <= TC:
            j1 += 1
        chunks.append((j0, j1, int(tile_off[j0]), int(tile_off[j1])))
        j0 = j1
    G_max = max(j1 - j0 for j0, j1, _, _ in chunks)
    T_max = int(T_ws.max())

    wo_arr = np.ascontiguousarray(
        Wo.reshape(H, D, comb).transpose(1, 0, 2)
    ).astype(BF16).reshape(D, H * comb)
    bo_col = bo.reshape(comb, 1).astype(np.float32)

    nc = _build_nc(NW, [int(x) for x in T_ws], chunks, G_max, T_max, comb)
    in_maps = []
    for c in range(N_CORES):
        in_maps.append({
            "ev": ev_all[c],
            "evt": evt_all[c],
            "cc": cc_all[c],
            "msk": msk_all[c],
            "wo": wo_arr,
            "bo": bo_col,
        })
    res = bass_utils.run_bass_kernel_spmd(
        nc, in_maps, core_ids=list(range(N_CORES)), trace=trace
    )
    full = np.zeros((N, comb), np.float32)
    for c in range(N_CORES):
        outc = np.asarray(res.results[c]["out"], np.float32).T  # [S_pad, comb]
        vm = valid[c].reshape(-1)
        sel = seg_ids[c].reshape(-1)[vm]
        full[sel] = outc[: vm.shape[0]][vm]
    # empty segments: reference = bias only (1/0 is undefined there)
    seg_cnt = np.bincount(ast_value.astype(np.int64), minlength=N)[:N]
    full[seg_cnt == 0] = bo[None, :]
    return full, res


def kernel(**inputs):
    ast = np.asarray(inputs["ast_nodes_encodings"], np.float32)
    Wq = np.asarray(inputs["Wq"], np.float32)
    bq = np.asarray(inputs["bq"], np.float32)
    Wk = np.asarray(inputs["Wk"], np.float32)
    bk = np.asarray(inputs["bk"], np.float32)  # cancels inside segment softmax
    Wo = np.asarray(inputs["Wo"], np.float32)
    bo = np.asarray(inputs["bo"], np.float32)
    ast_key = np.asarray(inputs["ast_key"]).astype(np.int64)
    ast_value = np.asarray(inputs["ast_value"]).astype(np.int64)
    pdg_key = np.asarray(inputs["pdg_key"]).astype(np.int64)
    pdg_value = np.asarray(inputs["pdg_value"]).astype(np.int64)
    N = int(np.asarray(inputs["nr_cfg_nodes"]))
    out, _ = _run(ast, Wq, bq, Wk, bk, Wo, bo,
                  ast_key, ast_value, pdg_key, pdg_value, N)
    return out
